# revision 7
# baseline (speedup 1.0000x reference)
"""GAT (3-layer, PyG-style) on 8 Trainium2 NeuronCores via Bass/Tile.

Sharding: nodes (and incident edges by dst) are partitioned across the 8
cores. Each core computes h = x @ W for its node shard (data-parallel), the
shards are AllGathered into a full per-core h table, and each core runs
segment-softmax attention + aggregation for the edges whose dst it owns
(edges presorted by dst into 128-edge tiles; aggregation is a one-hot
matmul into PSUM; messages are gathered from the full table by src index).
"""
import sys
import numpy as np

sys.path.insert(0, '/opt/trn_rl_repo')

import tile_patch  # noqa: F401  (walrus 1-sync-wait legalization)
import concourse.bass as bass
import concourse.mybir as mybir
import concourse.tile as tile
from concourse.bass_utils import run_bass_kernel_spmd
from concourse.masks import make_identity

# Problem constants (hardcoded per contract)
N = 50000
E = 600000
IN = 256
HID = 64
HEADS = 4
OUT = 64
NEG_SLOPE = 0.2

C = 8               # cores
P = 128             # partitions / tile edge rows
NPC = 6272          # nodes per core (49 * 128), padded
NPAD = C * NPC      # 50176
W = NPC // P        # 49 windows per core

F32 = mybir.dt.float32
I32 = mybir.dt.int32

# h_ext row formats
RC01 = 264          # layers 0,1: [h 256 | as 4 | ad 4]
RC2 = 66            # layer 2:    [h 64  | as 1 | ad 1]


def _host_prep(x, edge_index, Ws, as_, ad_, bs):
    """Build per-core shards + shared tile structure. All numpy."""
    rng_src = edge_index[0].astype(np.int64)
    rng_dst = edge_index[1].astype(np.int64)
    # self-loops for all padded nodes (pad nodes get one so deg>0, denom>0)
    loops = np.arange(NPAD, dtype=np.int64)
    src = np.concatenate([rng_src, loops])
    dst = np.concatenate([rng_dst, loops])
    deg = np.bincount(dst, minlength=NPAD).astype(np.float32)

    # per-core edge lists sorted by local dst
    core_of = dst // NPC
    dloc = dst % NPC
    order = np.lexsort((src, dloc, core_of))
    src, dst, dloc = src[order], dst[order], dloc[order]
    core_of = core_of[order]

    # tiles per window = max over cores (shared static program structure)
    win = dloc // P
    counts = np.zeros((C, W), np.int64)
    for c in range(C):
        m = core_of == c
        counts[c] = np.bincount(win[m], minlength=W)
    tiles_per_win = np.maximum(1, np.ceil(counts / P).astype(np.int64).max(axis=0))
    T_tot = int(tiles_per_win.sum())

    # per-core per-tile arrays: esrc [P, T_tot] i32, edloc [P, T_tot] f32,
    # edst [P, T_tot] i32 (global dst, safe value for pads)
    esrc = np.zeros((C, P, T_tot), np.int32)
    edloc = np.full((C, P, T_tot), -1.0, np.float32)
    edst = np.zeros((C, P, T_tot), np.int32)
    tile_win = np.zeros(T_tot, np.int32)      # window of each tile
    ti = 0
    win_tile_start = []
    for w in range(W):
        win_tile_start.append(ti)
        for _ in range(tiles_per_win[w]):
            tile_win[ti] = w
            ti += 1
    assert ti == T_tot

    for c in range(C):
        m = core_of == c
        s_c, dl_c, d_c = src[m], dloc[m], dst[m]
        w_c = dl_c // P
        for w in range(W):
            mw = w_c == w
            s_w, dl_w, d_w = s_c[mw], dl_c[mw], d_c[mw]
            n = len(s_w)
            t0 = win_tile_start[w]
            for t in range(tiles_per_win[w]):
                lo, hi = t * P, min((t + 1) * P, n)
                if lo >= n:
                    # fully padded tile: keep defaults (edst -> core base)
                    edst[c, :, t0 + t] = c * NPC
                    continue
                k = hi - lo
                esrc[c, :k, t0 + t] = s_w[lo:hi]
                edloc[c, :k, t0 + t] = (dl_w[lo:hi] - w * P).astype(np.float32)
                edst[c, :k, t0 + t] = d_w[lo:hi]
                if k < P:
                    edst[c, k:, t0 + t] = c * NPC

    # projection matrices per layer: Pm = [W | W@as | W@ad]
    def proj(Wl, a_s, a_d, heads, ch):
        Wr = Wl.reshape(Wl.shape[0], heads, ch)
        was = np.einsum('khc,hc->kh', Wr, a_s)
        wad = np.einsum('khc,hc->kh', Wr, a_d)
        return np.concatenate([Wl, was, wad], axis=1).astype(np.float32)

    Pm = [proj(Ws[0], as_[0], ad_[0], HEADS, HID),
          proj(Ws[1], as_[1], ad_[1], HEADS, HID),
          proj(Ws[2], as_[2], ad_[2], 1, OUT)]

    # layer-0 x^T shards [256, NPC], zero-padded
    xpad = np.zeros((NPAD, IN), np.float32)
    xpad[:N] = x
    xT0 = np.stack([xpad[c * NPC:(c + 1) * NPC].T.copy() for c in range(C)])

    # deg in window layout [P, W]: degw[p, w] = deg[base + w*P + p]
    degw = np.stack([deg[c * NPC:(c + 1) * NPC].reshape(W, P).T.copy()
                     for c in range(C)])

    # bias replicated across partitions
    biasr = [np.repeat(b[None, :], P, axis=0).astype(np.float32) for b in bs]

    struct = dict(tiles_per_win=tiles_per_win, tile_win=tile_win,
                  win_tile_start=win_tile_start, T_tot=T_tot)
    percore = []
    for c in range(C):
        percore.append(dict(
            xT0=xT0[c], esrc=esrc[c], edloc=edloc[c], edst=edst[c],
            degw=degw[c],
            Pm0=Pm[0], Pm1=Pm[1], Pm2=Pm[2],
            biasr0=biasr[0], biasr1=biasr[1], biasr2=biasr[2],
        ))
    return struct, percore


def _build_nc(struct):
    tiles_per_win = struct['tiles_per_win']
    win_tile_start = struct['win_tile_start']
    T_tot = struct['T_tot']

    nc = bass.Bass(target_bir_lowering=False)

    # I/O
    xT0 = nc.dram_tensor("xT0", [IN, NPC], F32, kind="ExternalInput")
    esrc = nc.dram_tensor("esrc", [P, T_tot], I32, kind="ExternalInput")
    edloc = nc.dram_tensor("edloc", [P, T_tot], F32, kind="ExternalInput")
    edst = nc.dram_tensor("edst", [P, T_tot], I32, kind="ExternalInput")
    degw = nc.dram_tensor("degw", [P, W], F32, kind="ExternalInput")
    Pm = [nc.dram_tensor(f"Pm{l}", [IN, RC01 if l < 2 else RC2], F32,
                         kind="ExternalInput") for l in range(3)]
    biasr = [nc.dram_tensor(f"biasr{l}", [P, IN if l < 2 else OUT], F32,
                            kind="ExternalInput") for l in range(3)]
    yout = nc.dram_tensor("yout", [NPC, OUT], F32, kind="ExternalOutput")

    with tile.TileContext(nc) as tc:
        with tc.tile_pool(name="dram", bufs=1, space="DRAM") as dram, \
             tc.tile_pool(name="const", bufs=1) as cpool, \
             tc.tile_pool(name="sbuf", bufs=3) as pool, \
             tc.tile_pool(name="gather", bufs=6) as gpool, \
             tc.tile_pool(name="psum", bufs=2, space="PSUM") as psum:

            # ---- constants in SBUF ----
            io32 = cpool.tile([P, P], I32)
            nc.gpsimd.iota(io32[:], pattern=[[1, P]], base=0, channel_multiplier=0)
            iota_f = cpool.tile([P, P], F32)
            nc.vector.tensor_copy(out=iota_f[:], in_=io32[:])
            ident = cpool.tile([P, P], F32)
            make_identity(nc, ident[:])
            esrc_sb = cpool.tile([P, T_tot], I32, name="esrc_sb")
            nc.sync.dma_start(out=esrc_sb[:], in_=esrc[:])
            edloc_sb = cpool.tile([P, T_tot], F32, name="edloc_sb")
            nc.sync.dma_start(out=edloc_sb[:], in_=edloc[:])
            edst_sb = cpool.tile([P, T_tot], I32, name="edst_sb")
            nc.sync.dma_start(out=edst_sb[:], in_=edst[:])
            degw_sb = cpool.tile([P, W], F32, name="degw_sb")
            nc.sync.dma_start(out=degw_sb[:], in_=degw[:])

            # per-layer DRAM tables
            ag_in = [dram.tile([NPC, RC01 if l < 2 else RC2], F32,
                               name=f"ag_in{l}") for l in range(3)]
            ag_out = [dram.tile([NPAD, RC01 if l < 2 else RC2], F32,
                                name=f"ag_out{l}", addr_space="Shared")
                      for l in range(3)]
            xT = [None,
                  dram.tile([IN, NPC], F32, name="xT1"),
                  dram.tile([IN, NPC], F32, name="xT2")]

            for l in range(3):
                RC = RC01 if l < 2 else RC2
                NH = HEADS if l < 2 else 1
                CH = HID if l < 2 else OUT
                NMSG = NH * CH + NH     # 260 / 65
                OCOL = NH * CH          # 256 / 64

                # ---- dense phase: h_ext = x @ Pm[l] (per window) ----
                pm_sb = [pool.tile([P, RC], F32, name=f"pm{l}_{k}", bufs=1)
                         for k in range(2)]
                for k in range(2):
                    nc.sync.dma_start(out=pm_sb[k][:], in_=Pm[l][k * P:(k + 1) * P, :])
                bias_sb = pool.tile([P, OCOL], F32, name=f"bias{l}", bufs=1)
                nc.sync.dma_start(out=bias_sb[:], in_=biasr[l][:])

                for w in range(W):
                    xt = [pool.tile([P, P], F32, name="xt", bufs=4) for _ in range(2)]
                    for k in range(2):
                        if l == 0:
                            nc.sync.dma_start(
                                out=xt[k][:], in_=xT0[k * P:(k + 1) * P,
                                                      w * P:(w + 1) * P])
                        else:
                            nc.sync.dma_start(
                                out=xt[k][:], in_=xT[l][k * P:(k + 1) * P,
                                                        w * P:(w + 1) * P])
                    ps_h = psum.tile([P, RC], F32, space="PSUM", name="ps_h")
                    for k in range(2):
                        nc.tensor.matmul(out=ps_h[:], lhsT=xt[k][:], rhs=pm_sb[k][:],
                                         start=(k == 0), stop=(k == 1))
                    hx = pool.tile([P, RC], F32, name="hx", bufs=4)
                    nc.vector.tensor_copy(out=hx[:], in_=ps_h[:])
                    nc.scalar.dma_start(out=ag_in[l][w * P:(w + 1) * P, :], in_=hx[:])

                # ---- exchange: AllGather shards -> full table ----
                nc.gpsimd.collective_compute(
                    "AllGather", mybir.AluOpType.bypass,
                    replica_groups=[list(range(C))],
                    ins=[ag_in[l].opt()], outs=[ag_out[l].opt()],
                )

                # ---- edge phase ----
                tbl = ag_out[l]
                for w in range(W):
                    Tw = int(tiles_per_win[w])
                    t0 = win_tile_start[w]
                    ps_agg = psum.tile([P, NMSG], F32, space="PSUM", name="ps_agg")
                    for t in range(Tw):
                        ti = t0 + t
                        g = gpool.tile([P, RC], F32, name="g")
                        nc.gpsimd.indirect_dma_start(
                            out=g[:], out_offset=None, in_=tbl[:],
                            in_offset=bass.IndirectOffsetOnAxis(
                                ap=esrc_sb[:, ti:ti + 1], axis=0))
                        adx = gpool.tile([P, NH], F32, name="adx")
                        nc.gpsimd.indirect_dma_start(
                            out=adx[:], out_offset=None, in_=tbl[:],
                            in_offset=bass.IndirectOffsetOnAxis(
                                ap=edst_sb[:, ti:ti + 1], axis=0),
                            element_offset=OCOL + NH)
                        # logits -> lrelu -> exp
                        msg = gpool.tile([P, NMSG], F32, name="msg")
                        wv = msg[:, OCOL:OCOL + NH]
                        nc.vector.tensor_add(out=wv, in0=g[:, OCOL:OCOL + NH],
                                             in1=adx[:])
                        # leaky-relu exactly on DVE (ACT Lrelu ignores alpha)
                        lrt = gpool.tile([P, NH], F32, name="lrt")
                        nc.vector.tensor_scalar_mul(out=lrt[:], in0=wv,
                                                    scalar1=NEG_SLOPE)
                        nc.vector.tensor_tensor(out=wv, in0=wv, in1=lrt[:],
                                                op=mybir.AluOpType.max)
                        nc.scalar.activation(out=wv, in_=wv,
                                             func=mybir.ActivationFunctionType.Exp)
                        # msg[:, :OCOL] = g[:, :OCOL] * w (per-head broadcast)
                        g3 = g[:, 0:OCOL].rearrange("p (h c) -> p h c", h=NH)
                        m3 = msg[:, 0:OCOL].rearrange("p (h c) -> p h c", h=NH)
                        wb = wv.unsqueeze(2).to_broadcast([P, NH, CH])
                        nc.vector.tensor_tensor(out=m3, in0=g3, in1=wb,
                                                op=mybir.AluOpType.mult)
                        # one-hot S
                        S = gpool.tile([P, P], F32, name="S")
                        nc.vector.tensor_tensor(
                            out=S[:], in0=iota_f[:],
                            in1=edloc_sb[:, ti:ti + 1].to_broadcast([P, P]),
                            op=mybir.AluOpType.is_equal)
                        nc.tensor.matmul(out=ps_agg[:], lhsT=S[:], rhs=msg[:],
                                         start=(t == 0), stop=(t == Tw - 1))

                    # ---- epilogue for window w ----
                    scale = pool.tile([P, NH], F32, name="scale", bufs=4)
                    nc.vector.tensor_tensor(
                        out=scale[:], in0=ps_agg[:, OCOL:OCOL + NH],
                        in1=degw_sb[:, w:w + 1].to_broadcast([P, NH]),
                        op=mybir.AluOpType.mult)
                    nc.vector.reciprocal(out=scale[:], in_=scale[:])
                    ov = pool.tile([P, OCOL], F32, name="ov", bufs=4)
                    o3 = ov[:].rearrange("p (h c) -> p h c", h=NH)
                    p3 = ps_agg[:, 0:OCOL].rearrange("p (h c) -> p h c", h=NH)
                    sb = scale[:].unsqueeze(2).to_broadcast([P, NH, CH])
                    nc.vector.tensor_tensor(out=o3, in0=p3, in1=sb,
                                            op=mybir.AluOpType.mult)
                    if l < 2:
                        nc.vector.tensor_add(out=ov[:], in0=ov[:], in1=bias_sb[:])
                        nc.vector.tensor_scalar_max(out=ov[:], in0=ov[:], scalar1=0.0)
                        # transpose to xT[l+1] via PE
                        for k in range(2):
                            ps_t = psum.tile([P, P], F32, space="PSUM", name="ps_t")
                            nc.tensor.transpose(out=ps_t[:],
                                                in_=ov[:, k * P:(k + 1) * P],
                                                identity=ident[:])
                            tx = pool.tile([P, P], F32, name="tx", bufs=4)
                            nc.vector.tensor_copy(out=tx[:], in_=ps_t[:])
                            nc.scalar.dma_start(
                                out=xT[l + 1][k * P:(k + 1) * P, w * P:(w + 1) * P],
                                in_=tx[:])
                    else:
                        nc.vector.tensor_add(out=ov[:], in0=ov[:], in1=bias_sb[:])
                        nc.scalar.dma_start(out=yout[w * P:(w + 1) * P, :], in_=ov[:])

    return nc


def kernel(x, edge_index, W0, a0s, a0d, b0, W1, a1s, a1d, b1, W2, a2s, a2d, b2,
           _return_nc=False):
    x = np.asarray(x, np.float32)
    edge_index = np.asarray(edge_index, np.int32)
    Ws = [np.asarray(W0, np.float32), np.asarray(W1, np.float32),
          np.asarray(W2, np.float32)]
    as_ = [np.asarray(a0s, np.float32), np.asarray(a1s, np.float32),
           np.asarray(a2s, np.float32)]
    ad_ = [np.asarray(a0d, np.float32), np.asarray(a1d, np.float32),
           np.asarray(a2d, np.float32)]
    bs = [np.asarray(b0, np.float32), np.asarray(b1, np.float32),
          np.asarray(b2, np.float32)]

    struct, percore = _host_prep(x, edge_index, Ws, as_, ad_, bs)
    nc = _build_nc(struct)
    if _return_nc:
        return nc, struct, percore
    res = run_bass_kernel_spmd(nc, [dict(pc) for pc in percore],
                               core_ids=list(range(C)))
    out = np.concatenate([res.results[c]["yout"] for c in range(C)], axis=0)
    return out[:N]


# revision 9
# speedup vs baseline: 1.0962x; 1.0962x over previous
"""GAT (3-layer, PyG-style) on 8 Trainium2 NeuronCores via Bass/Tile.

Sharding: nodes (and incident edges by dst) are partitioned across the 8
cores. Each core computes h = x @ W for its node shard (data-parallel), the
shards are AllGathered into a full per-core h table, and each core runs
segment-softmax attention + aggregation for the edges whose dst it owns
(edges presorted by dst into 128-edge tiles; aggregation is a one-hot
matmul into PSUM; messages are gathered from the full table by src index).
"""
import sys
import numpy as np

sys.path.insert(0, '/opt/trn_rl_repo')

import tile_patch  # noqa: F401  (walrus 1-sync-wait legalization)
import concourse.bass as bass
import concourse.mybir as mybir
import concourse.tile as tile
from concourse.bass_utils import run_bass_kernel_spmd
from concourse.masks import make_identity

# Problem constants (hardcoded per contract)
N = 50000
E = 600000
IN = 256
HID = 64
HEADS = 4
OUT = 64
NEG_SLOPE = 0.2

C = 8               # cores
P = 128             # partitions / tile edge rows
NPC = 6272          # nodes per core (49 * 128), padded
NPAD = C * NPC      # 50176
W = NPC // P        # 49 windows per core

F32 = mybir.dt.float32
F32R = mybir.dt.float32r
BF16 = mybir.dt.bfloat16
I32 = mybir.dt.int32
TBL_DT = BF16          # dtype of exchanged/gathered h_ext tables + messages
MM_CAST = True         # bitcast f32 matmul operands to float32r

# h_ext row formats
RC01 = 264          # layers 0,1: [h 256 | as 4 | ad 4]
RC2 = 66            # layer 2:    [h 64  | as 1 | ad 1]


def _host_prep(x, edge_index, Ws, as_, ad_, bs):
    """Build per-core shards + shared tile structure. All numpy."""
    rng_src = edge_index[0].astype(np.int64)
    rng_dst = edge_index[1].astype(np.int64)
    # self-loops for all padded nodes (pad nodes get one so deg>0, denom>0)
    loops = np.arange(NPAD, dtype=np.int64)
    src = np.concatenate([rng_src, loops])
    dst = np.concatenate([rng_dst, loops])
    deg = np.bincount(dst, minlength=NPAD).astype(np.float32)

    # per-core edge lists sorted by local dst
    core_of = dst // NPC
    dloc = dst % NPC
    order = np.lexsort((src, dloc, core_of))
    src, dst, dloc = src[order], dst[order], dloc[order]
    core_of = core_of[order]

    # tiles per window = max over cores (shared static program structure)
    win = dloc // P
    counts = np.zeros((C, W), np.int64)
    for c in range(C):
        m = core_of == c
        counts[c] = np.bincount(win[m], minlength=W)
    tiles_per_win = np.maximum(1, np.ceil(counts / P).astype(np.int64).max(axis=0))
    T_tot = int(tiles_per_win.sum())

    # per-core per-tile arrays: esrc [P, T_tot] i32, edloc [P, T_tot] f32,
    # edst [P, T_tot] i32 (global dst, safe value for pads)
    esrc = np.zeros((C, P, T_tot), np.int32)
    edloc = np.full((C, P, T_tot), -1.0, np.float32)
    edst = np.zeros((C, P, T_tot), np.int32)
    tile_win = np.zeros(T_tot, np.int32)      # window of each tile
    ti = 0
    win_tile_start = []
    for w in range(W):
        win_tile_start.append(ti)
        for _ in range(tiles_per_win[w]):
            tile_win[ti] = w
            ti += 1
    assert ti == T_tot

    for c in range(C):
        m = core_of == c
        s_c, dl_c, d_c = src[m], dloc[m], dst[m]
        w_c = dl_c // P
        for w in range(W):
            mw = w_c == w
            s_w, dl_w, d_w = s_c[mw], dl_c[mw], d_c[mw]
            n = len(s_w)
            t0 = win_tile_start[w]
            for t in range(tiles_per_win[w]):
                lo, hi = t * P, min((t + 1) * P, n)
                if lo >= n:
                    # fully padded tile: keep defaults (edst -> core base)
                    edst[c, :, t0 + t] = c * NPC
                    continue
                k = hi - lo
                esrc[c, :k, t0 + t] = s_w[lo:hi]
                edloc[c, :k, t0 + t] = (dl_w[lo:hi] - w * P).astype(np.float32)
                edst[c, :k, t0 + t] = d_w[lo:hi]
                if k < P:
                    edst[c, k:, t0 + t] = c * NPC

    # projection matrices per layer: Pm = [W | W@as | W@ad]
    def proj(Wl, a_s, a_d, heads, ch):
        Wr = Wl.reshape(Wl.shape[0], heads, ch)
        was = np.einsum('khc,hc->kh', Wr, a_s)
        wad = np.einsum('khc,hc->kh', Wr, a_d)
        return np.concatenate([Wl, was, wad], axis=1).astype(np.float32)

    Pm = [proj(Ws[0], as_[0], ad_[0], HEADS, HID),
          proj(Ws[1], as_[1], ad_[1], HEADS, HID),
          proj(Ws[2], as_[2], ad_[2], 1, OUT)]

    # layer-0 x^T shards [256, NPC], zero-padded
    xpad = np.zeros((NPAD, IN), np.float32)
    xpad[:N] = x
    xT0 = np.stack([xpad[c * NPC:(c + 1) * NPC].T.copy() for c in range(C)])

    # deg in window layout [P, W]: degw[p, w] = deg[base + w*P + p]
    degw = np.stack([deg[c * NPC:(c + 1) * NPC].reshape(W, P).T.copy()
                     for c in range(C)])

    # bias replicated across partitions
    biasr = [np.repeat(b[None, :], P, axis=0).astype(np.float32) for b in bs]

    struct = dict(tiles_per_win=tiles_per_win, tile_win=tile_win,
                  win_tile_start=win_tile_start, T_tot=T_tot)
    percore = []
    for c in range(C):
        percore.append(dict(
            xT0=xT0[c], esrc=esrc[c], edloc=edloc[c], edst=edst[c],
            degw=degw[c],
            Pm0=Pm[0], Pm1=Pm[1], Pm2=Pm[2],
            biasr0=biasr[0], biasr1=biasr[1], biasr2=biasr[2],
        ))
    return struct, percore


def _r(ap):
    return ap.bitcast(F32R) if MM_CAST else ap


def _build_nc(struct):
    tiles_per_win = struct['tiles_per_win']
    win_tile_start = struct['win_tile_start']
    T_tot = struct['T_tot']

    nc = bass.Bass(target_bir_lowering=False)

    # I/O
    xT0 = nc.dram_tensor("xT0", [IN, NPC], F32R, kind="ExternalInput")
    esrc = nc.dram_tensor("esrc", [P, T_tot], I32, kind="ExternalInput")
    edloc = nc.dram_tensor("edloc", [P, T_tot], F32, kind="ExternalInput")
    edst = nc.dram_tensor("edst", [P, T_tot], I32, kind="ExternalInput")
    degw = nc.dram_tensor("degw", [P, W], F32, kind="ExternalInput")
    Pm = [nc.dram_tensor(f"Pm{l}", [IN, RC01 if l < 2 else RC2], F32R,
                         kind="ExternalInput") for l in range(3)]
    biasr = [nc.dram_tensor(f"biasr{l}", [P, IN if l < 2 else OUT], F32,
                            kind="ExternalInput") for l in range(3)]
    yout = nc.dram_tensor("yout", [NPC, OUT], F32, kind="ExternalOutput")

    with tile.TileContext(nc) as tc:
        with tc.tile_pool(name="dram", bufs=1, space="DRAM") as dram, \
             tc.tile_pool(name="const", bufs=1) as cpool, \
             tc.tile_pool(name="sbuf", bufs=3) as pool, \
             tc.tile_pool(name="gather", bufs=6) as gpool, \
             tc.tile_pool(name="psum", bufs=2, space="PSUM") as psum:

            # ---- constants in SBUF ----
            io32 = cpool.tile([P, P], I32)
            nc.gpsimd.iota(io32[:], pattern=[[1, P]], base=0, channel_multiplier=0)
            iota_f = cpool.tile([P, P], F32)
            nc.vector.tensor_copy(out=iota_f[:], in_=io32[:])
            ident = cpool.tile([P, P], F32)
            make_identity(nc, ident[:])
            esrc_sb = cpool.tile([P, T_tot], I32, name="esrc_sb")
            nc.sync.dma_start(out=esrc_sb[:], in_=esrc[:])
            edloc_sb = cpool.tile([P, T_tot], F32, name="edloc_sb")
            nc.sync.dma_start(out=edloc_sb[:], in_=edloc[:])
            edst_sb = cpool.tile([P, T_tot], I32, name="edst_sb")
            nc.sync.dma_start(out=edst_sb[:], in_=edst[:])
            degw_sb = cpool.tile([P, W], F32, name="degw_sb")
            nc.sync.dma_start(out=degw_sb[:], in_=degw[:])

            # per-layer DRAM tables
            ag_in = [dram.tile([NPC, RC01 if l < 2 else RC2], TBL_DT,
                               name=f"ag_in{l}") for l in range(3)]
            ag_out = [dram.tile([NPAD, RC01 if l < 2 else RC2], TBL_DT,
                                name=f"ag_out{l}", addr_space="Shared")
                      for l in range(3)]
            xT = [None,
                  dram.tile([IN, NPC], F32R, name="xT1"),
                  dram.tile([IN, NPC], F32R, name="xT2")]

            for l in range(3):
                RC = RC01 if l < 2 else RC2
                NH = HEADS if l < 2 else 1
                CH = HID if l < 2 else OUT
                NMSG = NH * CH + NH     # 260 / 65
                OCOL = NH * CH          # 256 / 64

                # ---- dense phase: h_ext = x @ Pm[l] (per window) ----
                pm_sb = [pool.tile([P, RC], F32R, name=f"pm{l}_{k}", bufs=1)
                         for k in range(2)]
                for k in range(2):
                    nc.sync.dma_start(out=pm_sb[k][:], in_=Pm[l][k * P:(k + 1) * P, :])
                bias_sb = pool.tile([P, OCOL], F32, name=f"bias{l}", bufs=1)
                nc.sync.dma_start(out=bias_sb[:], in_=biasr[l][:])

                for w in range(W):
                    xt = [pool.tile([P, P], F32R, name="xt", bufs=4) for _ in range(2)]
                    for k in range(2):
                        if l == 0:
                            nc.sync.dma_start(
                                out=xt[k][:], in_=xT0[k * P:(k + 1) * P,
                                                      w * P:(w + 1) * P])
                        else:
                            nc.sync.dma_start(
                                out=xt[k][:], in_=xT[l][k * P:(k + 1) * P,
                                                        w * P:(w + 1) * P])
                    ps_h = psum.tile([P, RC], F32, space="PSUM", name="ps_h")
                    for k in range(2):
                        nc.tensor.matmul(out=ps_h[:], lhsT=xt[k][:],
                                         rhs=pm_sb[k][:],
                                         start=(k == 0), stop=(k == 1))
                    hx = pool.tile([P, RC], TBL_DT, name="hx", bufs=4)
                    nc.vector.tensor_copy(out=hx[:], in_=ps_h[:])
                    nc.scalar.dma_start(out=ag_in[l][w * P:(w + 1) * P, :], in_=hx[:])

                # ---- exchange: AllGather shards -> full table ----
                nc.gpsimd.collective_compute(
                    "AllGather", mybir.AluOpType.bypass,
                    replica_groups=[list(range(C))],
                    ins=[ag_in[l].opt()], outs=[ag_out[l].opt()],
                )

                # ---- edge phase ----
                tbl = ag_out[l]
                for w in range(W):
                    Tw = int(tiles_per_win[w])
                    t0 = win_tile_start[w]
                    ps_agg = psum.tile([P, NMSG], F32, space="PSUM", name="ps_agg")
                    for t in range(Tw):
                        ti = t0 + t
                        g = gpool.tile([P, RC], TBL_DT, name="g")
                        nc.gpsimd.indirect_dma_start(
                            out=g[:], out_offset=None, in_=tbl[:],
                            in_offset=bass.IndirectOffsetOnAxis(
                                ap=esrc_sb[:, ti:ti + 1], axis=0))
                        adx = gpool.tile([P, NH], TBL_DT, name="adx")
                        nc.gpsimd.indirect_dma_start(
                            out=adx[:], out_offset=None, in_=tbl[:],
                            in_offset=bass.IndirectOffsetOnAxis(
                                ap=edst_sb[:, ti:ti + 1], axis=0),
                            element_offset=OCOL + NH)
                        # logits -> lrelu -> exp
                        msg = gpool.tile([P, NMSG], TBL_DT, name="msg")
                        wv = msg[:, OCOL:OCOL + NH]
                        nc.vector.tensor_add(out=wv, in0=g[:, OCOL:OCOL + NH],
                                             in1=adx[:])
                        # leaky-relu exactly on DVE (ACT Lrelu ignores alpha)
                        lrt = gpool.tile([P, NH], TBL_DT, name="lrt")
                        nc.vector.tensor_scalar_mul(out=lrt[:], in0=wv,
                                                    scalar1=NEG_SLOPE)
                        nc.vector.tensor_tensor(out=wv, in0=wv, in1=lrt[:],
                                                op=mybir.AluOpType.max)
                        nc.scalar.activation(out=wv, in_=wv,
                                             func=mybir.ActivationFunctionType.Exp)
                        # msg[:, :OCOL] = g[:, :OCOL] * w (per-head broadcast)
                        g3 = g[:, 0:OCOL].rearrange("p (h c) -> p h c", h=NH)
                        m3 = msg[:, 0:OCOL].rearrange("p (h c) -> p h c", h=NH)
                        wb = wv.unsqueeze(2).to_broadcast([P, NH, CH])
                        nc.vector.tensor_tensor(out=m3, in0=g3, in1=wb,
                                                op=mybir.AluOpType.mult)
                        # one-hot S
                        S = gpool.tile([P, P], TBL_DT, name="S")
                        nc.vector.tensor_tensor(
                            out=S[:], in0=iota_f[:],
                            in1=edloc_sb[:, ti:ti + 1].to_broadcast([P, P]),
                            op=mybir.AluOpType.is_equal)
                        nc.tensor.matmul(out=ps_agg[:], lhsT=S[:], rhs=msg[:],
                                         start=(t == 0), stop=(t == Tw - 1))

                    # ---- epilogue for window w ----
                    scale = pool.tile([P, NH], F32, name="scale", bufs=4)
                    nc.vector.tensor_tensor(
                        out=scale[:], in0=ps_agg[:, OCOL:OCOL + NH],
                        in1=degw_sb[:, w:w + 1].to_broadcast([P, NH]),
                        op=mybir.AluOpType.mult)
                    nc.vector.reciprocal(out=scale[:], in_=scale[:])
                    ov = pool.tile([P, OCOL], F32, name="ov", bufs=4)
                    o3 = ov[:].rearrange("p (h c) -> p h c", h=NH)
                    p3 = ps_agg[:, 0:OCOL].rearrange("p (h c) -> p h c", h=NH)
                    sb = scale[:].unsqueeze(2).to_broadcast([P, NH, CH])
                    nc.vector.tensor_tensor(out=o3, in0=p3, in1=sb,
                                            op=mybir.AluOpType.mult)
                    if l < 2:
                        nc.vector.tensor_add(out=ov[:], in0=ov[:], in1=bias_sb[:])
                        nc.vector.tensor_scalar_max(out=ov[:], in0=ov[:], scalar1=0.0)
                        # transpose to xT[l+1] via PE
                        for k in range(2):
                            ps_t = psum.tile([P, P], F32, space="PSUM", name="ps_t")
                            nc.tensor.transpose(out=ps_t[:],
                                                in_=ov[:, k * P:(k + 1) * P],
                                                identity=ident[:])
                            tx = pool.tile([P, P], F32R, name="tx", bufs=4)
                            nc.vector.tensor_copy(out=tx[:], in_=ps_t[:])
                            nc.scalar.dma_start(
                                out=xT[l + 1][k * P:(k + 1) * P, w * P:(w + 1) * P],
                                in_=tx[:])
                    else:
                        nc.vector.tensor_add(out=ov[:], in0=ov[:], in1=bias_sb[:])
                        nc.scalar.dma_start(out=yout[w * P:(w + 1) * P, :], in_=ov[:])

    return nc


def kernel(x, edge_index, W0, a0s, a0d, b0, W1, a1s, a1d, b1, W2, a2s, a2d, b2,
           _return_nc=False):
    x = np.asarray(x, np.float32)
    edge_index = np.asarray(edge_index, np.int32)
    Ws = [np.asarray(W0, np.float32), np.asarray(W1, np.float32),
          np.asarray(W2, np.float32)]
    as_ = [np.asarray(a0s, np.float32), np.asarray(a1s, np.float32),
           np.asarray(a2s, np.float32)]
    ad_ = [np.asarray(a0d, np.float32), np.asarray(a1d, np.float32),
           np.asarray(a2d, np.float32)]
    bs = [np.asarray(b0, np.float32), np.asarray(b1, np.float32),
          np.asarray(b2, np.float32)]

    struct, percore = _host_prep(x, edge_index, Ws, as_, ad_, bs)
    nc = _build_nc(struct)
    if _return_nc:
        return nc, struct, percore
    res = run_bass_kernel_spmd(nc, [dict(pc) for pc in percore],
                               core_ids=list(range(C)))
    out = np.concatenate([res.results[c]["yout"] for c in range(C)], axis=0)
    return out[:N]
